# revision 4
# baseline (speedup 1.0000x reference)
"""Event-driven SSM layer (LIF spiking scan) on 8 TRN2 NeuronCores.

Sharding: data-parallel over batch (B=8 -> 1 batch/core). Per-core scan runs
the 32-step LIF recurrence on [S=256] rows in transposed (channel-major)
layout. Adaptive thresholds need a global spike-mean per step -> one fused
AllReduce of a [128,5] f32 count tile per step.

Math notes:
 - anti-spikes ns = (v < thr) are computed instead of spikes; h = 1 - ns is
   folded into the matmuls via row-sum constants (PSUM preload bias) and
   negated A/C weights. Host flips the output at the end.
 - x@D.T, x@B.T, A@h, C@h run as bf16 hi/lo split matmuls (3 products for
   the x contractions, 2 for the binary-rhs A/C) -> ~1e-4 absolute accuracy.
 - membrane update v = decay*sv + update forms directly in PSUM: ScalarE
   preloads decay*sv + rowsum consts, matmuls accumulate on top
   (start=False; has_written bits pre-set by one zero-matmul per PSUM slot).
"""
import numpy as np
import ml_dtypes

B_, T_FULL, S, DM, DS = 8, 32, 256, 512, 64
KC, MC = DM // 128, DM // 128  # 4, 4
N_CORES = 8
ROWS_GLOBAL = float(B_ * S)
DECAY = float(np.float32(np.exp(np.float64(-1.0 / 2.0))))
ADAPT, BASE_THR, TGT = 0.1, 1.0, 0.1

bf16 = ml_dtypes.bfloat16


def _split(a):
    hi = a.astype(bf16)
    lo = (a - hi.astype(np.float32)).astype(bf16)
    return hi, lo


def _build(T):
    from concourse import bacc, mybir, tile

    nc = bacc.Bacc("TRN2", target_bir_lowering=False, debug=False,
                   num_devices=N_CORES)
    f32, bft = mybir.dt.float32, mybir.dt.bfloat16

    def din(name, shape, dt=bft):
        return nc.dram_tensor(name, shape, dt, kind="ExternalInput").ap()

    xhi_d = din("xhi", [T, KC, 128, S])
    xlo_d = din("xlo", [T, KC, 128, S])
    dthi_d = din("dthi", [KC, 128, DM])
    dtlo_d = din("dtlo", [KC, 128, DM])
    bthi_d = din("bthi", [KC, 128, DS])
    btlo_d = din("btlo", [KC, 128, DS])
    nathi_d = din("nathi", [DS, DS])
    natlo_d = din("natlo", [DS, DS])
    ncthi_d = din("ncthi", [DS, DM])
    nctlo_d = din("nctlo", [DS, DM])
    rsa_d = din("rsa", [DS, 1], f32)
    rsc_d = din("rsc", [128, MC], f32)
    out_d = nc.dram_tensor("out", [T, MC, 128, S], bft, kind="ExternalOutput").ap()

    CNT_COLS = MC + 1  # 4 output-chunk count columns + 1 state column
    c_upd = -ADAPT / ROWS_GLOBAL
    b_upd = ADAPT * (1.0 - TGT)

    with tile.TileContext(nc) as tc:
        with tc.tile_pool(name="w", bufs=1) as wp, \
             tc.tile_pool(name="st", bufs=1) as stp, \
             tc.tile_pool(name="io", bufs=3) as iop, \
             tc.tile_pool(name="sm", bufs=2) as smp, \
             tc.tile_pool(name="ps", bufs=2, space="PSUM") as psp, \
             tc.tile_pool(name="dr", bufs=1, space="DRAM") as drp:

            # ---------- persistent weights ----------
            dthi = [wp.tile([128, DM], bft, name=f"dthi{k}") for k in range(KC)]
            dtlo = [wp.tile([128, DM], bft, name=f"dtlo{k}") for k in range(KC)]
            bthi = [wp.tile([128, DS], bft, name=f"bthi{k}") for k in range(KC)]
            btlo = [wp.tile([128, DS], bft, name=f"btlo{k}") for k in range(KC)]
            nathi = wp.tile([DS, DS], bft, name="nathi")
            natlo = wp.tile([DS, DS], bft, name="natlo")
            ncthi = wp.tile([DS, DM], bft, name="ncthi")
            nctlo = wp.tile([DS, DM], bft, name="nctlo")
            rsa = wp.tile([DS, 1], f32, name="rsa")
            rsc = wp.tile([128, MC], f32, name="rsc")
            zl = wp.tile([1, 128], bft, name="zl")
            zr = wp.tile([1, 512], bft, name="zr")

            for k in range(KC):
                nc.sync.dma_start(out=dthi[k][:, :], in_=dthi_d[k])
                nc.sync.dma_start(out=dtlo[k][:, :], in_=dtlo_d[k])
                nc.sync.dma_start(out=bthi[k][:, :], in_=bthi_d[k])
                nc.sync.dma_start(out=btlo[k][:, :], in_=btlo_d[k])
            nc.sync.dma_start(out=nathi[:, :], in_=nathi_d[:, :])
            nc.sync.dma_start(out=natlo[:, :], in_=natlo_d[:, :])
            nc.sync.dma_start(out=ncthi[:, :], in_=ncthi_d[:, :])
            nc.sync.dma_start(out=nctlo[:, :], in_=nctlo_d[:, :])
            nc.sync.dma_start(out=rsa[:, :], in_=rsa_d[:, :])
            nc.sync.dma_start(out=rsc[:, :], in_=rsc_d[:, :])
            nc.gpsimd.memset(zl[:, :], 0.0)
            nc.gpsimd.memset(zr[:, :], 0.0)

            # ---------- persistent state ----------
            sv = stp.tile([DS, S], f32, name="sv")
            ov = stp.tile([128, MC * S], f32, name="ov")
            thr = stp.tile([128, CNT_COLS], f32, name="thr")
            nc.vector.memset(sv[:, :], 0.0)
            nc.vector.memset(ov[:, :], 0.0)
            nc.vector.memset(thr[:, :], BASE_THR)

            # ---------- AR dram buffers ----------
            ari = [drp.tile([128, CNT_COLS], f32, name=f"ari{t}") for t in range(T)]
            aro = [drp.tile([128, CNT_COLS], f32, name=f"aro{t}",
                            addr_space="Shared") for t in range(T)]

            # ---------- pre-set has_written bits on every PSUM slot ----------
            for i in range(2):
                pss = psp.tile([DS, S], f32, name=f"z_pss{i}", tag="pss")
                nc.tensor.matmul(pss[:, :], lhsT=zl[:, :DS], rhs=zr[:, :S],
                                 start=True, stop=True, skip_group_check=True)
                po = psp.tile([128, MC * S], f32, name=f"z_po{i}", tag="po")
                for h in range(2):
                    nc.tensor.matmul(po[:, h * 512:(h + 1) * 512],
                                     lhsT=zl[:, :], rhs=zr[:, :],
                                     start=True, stop=True, skip_group_check=True)

            nh_prev = None
            for t in range(T):
                # ---- stream x_t (hi/lo, 4 chunks each) ----
                xh = iop.tile([128, KC * S], bft, name=f"xh{t}", tag="xh")
                xl = iop.tile([128, KC * S], bft, name=f"xl{t}", tag="xl")
                for k in range(KC):
                    nc.sync.dma_start(out=xh[:, k * S:(k + 1) * S], in_=xhi_d[t, k])
                    nc.sync.dma_start(out=xl[:, k * S:(k + 1) * S], in_=xlo_d[t, k])

                # ---- state stage ----
                pss = psp.tile([DS, S], f32, name=f"pss{t}", tag="pss")
                nc.scalar.activation(
                    pss[:, :], sv[:, :], mybir.ActivationFunctionType.Identity,
                    bias=(0.0 if t == 0 else rsa[:, 0:1]), scale=DECAY)
                for k in range(KC):
                    xhk, xlk = xh[:, k * S:(k + 1) * S], xl[:, k * S:(k + 1) * S]
                    nc.tensor.matmul(pss[:, :], lhsT=bthi[k][:, :], rhs=xhk,
                                     start=False, stop=False, skip_group_check=True)
                    nc.tensor.matmul(pss[:, :], lhsT=bthi[k][:, :], rhs=xlk,
                                     start=False, stop=False, skip_group_check=True)
                    nc.tensor.matmul(pss[:, :], lhsT=btlo[k][:, :], rhs=xhk,
                                     start=False, stop=False, skip_group_check=True)
                if t > 0:
                    nc.tensor.matmul(pss[:, :], lhsT=nathi[:, :], rhs=nh_prev[:, :],
                                     start=False, stop=False, skip_group_check=True)
                    nc.tensor.matmul(pss[:, :], lhsT=natlo[:, :], rhs=nh_prev[:, :],
                                     start=False, stop=True, skip_group_check=True)

                cnt = smp.tile([128, CNT_COLS], f32, name=f"cnt{t}", tag="cnt")
                nc.gpsimd.memset(cnt[DS:128, MC:MC + 1], 0.0)
                nh = smp.tile([DS, S], bft, name=f"nh{t}", tag="nh")
                # anti-spike + count; then membrane reset sv = v * ns
                nc.vector.tensor_scalar(
                    nh[:, :], pss[:, :], thr[0:DS, MC:MC + 1], None,
                    mybir.AluOpType.is_lt, mybir.AluOpType.add,
                    accum_out=cnt[0:DS, MC:MC + 1])
                nc.vector.tensor_tensor(out=sv[:, :], in0=pss[:, :], in1=nh[:, :],
                                        op=mybir.AluOpType.mult)

                # ---- output stage ----
                po = psp.tile([128, MC * S], f32, name=f"po{t}", tag="po")
                for m in range(MC):
                    nc.scalar.activation(
                        po[:, m * S:(m + 1) * S], ov[:, m * S:(m + 1) * S],
                        mybir.ActivationFunctionType.Identity,
                        bias=rsc[:, m:m + 1], scale=DECAY)
                for m in range(MC):
                    pom = po[:, m * S:(m + 1) * S]
                    for k in range(KC):
                        xhk, xlk = xh[:, k * S:(k + 1) * S], xl[:, k * S:(k + 1) * S]
                        dh = dthi[k][:, m * 128:(m + 1) * 128]
                        dl = dtlo[k][:, m * 128:(m + 1) * 128]
                        nc.tensor.matmul(pom, lhsT=dh, rhs=xhk,
                                         start=False, stop=False, skip_group_check=True)
                        nc.tensor.matmul(pom, lhsT=dh, rhs=xlk,
                                         start=False, stop=False, skip_group_check=True)
                        nc.tensor.matmul(pom, lhsT=dl, rhs=xhk,
                                         start=False, stop=False, skip_group_check=True)
                for m in range(MC):
                    pom = po[:, m * S:(m + 1) * S]
                    nc.tensor.matmul(pom, lhsT=ncthi[:, m * 128:(m + 1) * 128],
                                     rhs=nh[:, :], start=False, stop=False,
                                     skip_group_check=True)
                    nc.tensor.matmul(pom, lhsT=nctlo[:, m * 128:(m + 1) * 128],
                                     rhs=nh[:, :], start=False, stop=True,
                                     skip_group_check=True)

                ns = smp.tile([128, MC * S], bft, name=f"ns{t}", tag="ns")
                for m in range(MC):
                    nc.vector.tensor_scalar(
                        ns[:, m * S:(m + 1) * S], po[:, m * S:(m + 1) * S],
                        thr[:, m:m + 1], None,
                        mybir.AluOpType.is_lt, mybir.AluOpType.add,
                        accum_out=cnt[:, m:m + 1])
                for m in range(MC):
                    nc.vector.tensor_tensor(
                        out=ov[:, m * S:(m + 1) * S], in0=po[:, m * S:(m + 1) * S],
                        in1=ns[:, m * S:(m + 1) * S], op=mybir.AluOpType.mult)

                for m in range(MC):
                    nc.scalar.dma_start(out=out_d[t, m], in_=ns[:, m * S:(m + 1) * S])

                # ---- fused threshold all-reduce ----
                nc.gpsimd.dma_start(out=ari[t][:, :], in_=cnt[:, :])
                nc.gpsimd.collective_compute(
                    "AllReduce", mybir.AluOpType.add,
                    replica_groups=[list(range(N_CORES))],
                    ins=[ari[t][:, :]], outs=[aro[t][:, :]])
                gs = smp.tile([128, CNT_COLS], f32, name=f"gs{t}", tag="gs")
                dl_t = smp.tile([128, CNT_COLS], f32, name=f"dl{t}", tag="dl")
                nc.gpsimd.dma_start(out=gs[:, :], in_=aro[t][:, :])
                nc.vector.tensor_scalar(dl_t[:, :], gs[:, :], c_upd, b_upd,
                                        mybir.AluOpType.mult, mybir.AluOpType.add)
                nc.vector.tensor_tensor(out=thr[:, :], in0=thr[:, :],
                                        in1=dl_t[:, :], op=mybir.AluOpType.add)

                nh_prev = nh

    nc.compile()
    return nc


_NC_CACHE = {}


def kernel(x, A, B, C, D, T=None):
    from concourse.bass_utils import run_bass_kernel_spmd

    x = np.asarray(x, dtype=np.float32)
    A = np.asarray(A, dtype=np.float32)
    B = np.asarray(B, dtype=np.float32)
    C = np.asarray(C, dtype=np.float32)
    D = np.asarray(D, dtype=np.float32)
    T = T or x.shape[1]

    if T not in _NC_CACHE:
        _NC_CACHE[T] = _build(T)
    nc = _NC_CACHE[T]

    dthi, dtlo = _split(D.T.reshape(KC, 128, DM))
    bthi, btlo = _split(B.T.reshape(KC, 128, DS))
    nathi, natlo = _split((-A).T.copy())
    ncthi, nctlo = _split((-C).T.copy())
    rsa = A.sum(axis=1, dtype=np.float32).reshape(DS, 1)
    rsc = C.sum(axis=1, dtype=np.float32).reshape(MC, 128).T.copy()

    shared = dict(dthi=dthi, dtlo=dtlo, bthi=bthi, btlo=btlo,
                  nathi=nathi, natlo=natlo, ncthi=ncthi, nctlo=nctlo,
                  rsa=rsa, rsc=rsc)

    in_maps = []
    for b in range(N_CORES):
        xt = np.ascontiguousarray(x[b, :T].transpose(0, 2, 1))  # [T, DM, S]
        xhi, xlo = _split(xt.reshape(T, KC, 128, S))
        in_maps.append({"xhi": xhi, "xlo": xlo, **shared})

    res = run_bass_kernel_spmd(nc, in_maps, core_ids=list(range(N_CORES)),
                               trace=bool(__import__("os").environ.get("KTRACE")))
    kernel.last_result = res

    out = np.empty((B_, T, S, DM), dtype=np.float32)
    for b in range(N_CORES):
        ns = res.results[b]["out"].astype(np.float32)  # [T, MC, 128, S]
        out[b] = (1.0 - ns).reshape(T, DM, S).transpose(0, 2, 1)
    return out
